# revision 28
# baseline (speedup 1.0000x reference)
"""DCRNN (2x GCNConv + GRU-over-nodes + Linear) on 8 Trainium2 cores, v3.

Strategy
--------
* Dense normalized adjacency (A+I, D^-1/2 scaled) sharded column-wise
  (1250 own cols + 64-col left halo per core), fp16, laid out
  column-chunk-major (3 chunks x 79 K-tiles, streamed in 8-tile groups
  with 7KB descriptor rows) so each layer's output chunk finishes as
  soon as its part of the stream has been consumed.
* GCN1 uses associativity: h1 = relu((A^T_strip.T @ x) @ W1 + b1) -- the
  big A matmul runs at M=64 (one matmul per K-tile) and there is no
  separate x@W1 pass.
* Per GCN1 chunk: xw2 = h1@W2 for the chunk's own rows, bounced to DRAM
  and AllGathered (3 chunked collectives pipelined behind the A stream).
  GCN2 consumes the gathered xw2 as stationary operands, chunk-major.
  Chunk 0 is streamed first and retained in SBUF; GCN2-chunk-0 matmuls
  are ordered by gather arrival and interleaved into GCN1's DMA-paced
  windows, so by the time the stream ends most of GCN2-c0 is done.
* GRU over the node sequence: 8 Jacobi fixed-point sweeps; gates from the
  previous sweep's h (double-buffered hidden state so the exact affine
  scan -- tensor_tensor_scan, chunked with per-partition carries --
  pipelines against the next chunk's matmuls); sweep 0 is matmul-free.
* Final Linear on the own rows; host concatenates the 8 shards.
"""

import numpy as np

NUM_NODES = 10000
IN_FEAT = 64
HID = 256
OUT = 3
CORES = 8
ROWS = NUM_NODES // CORES          # 1250
HALO = 64
L = ROWS + HALO                    # 1314 local sequence length
CW = L // 3                        # 438 chunk width
NCH = 3
KP = 128
KT = (NUM_NODES + KP - 1) // KP    # 79 K-tiles (global node tiles)
NKPAD = 80                         # K-tiles padded to a multiple of 8
GSZ = 8                            # K-tiles per stream group
NGRP = NKPAD // GSZ                # 10 stream groups per chunk
SWEEPS = 8
# own-node ranges per chunk (L-cols [0,64) are halo)
OWN_SPLIT = [0, CW - HALO, 2 * CW - HALO, ROWS]     # [0, 374, 812, 1250]

_CACHE = {}


def _chunks(total, step=512):
    return [(c, min(c + step, total)) for c in range(0, total, step)]


def _own_pieces(c):
    """<=128-wide pieces of chunk c's own-node range: (rowoff, width, lcol0).

    rowoff is relative to the chunk's bounce buffer; lcol0 indexes h1t's
    L axis (own node n lives at L-col HALO+n).
    """
    n0, n1 = OWN_SPLIT[c], OWN_SPLIT[c + 1]
    pieces = []
    n = n0
    while n < n1:
        w = min(KP, n1 - n)
        pieces.append((n - n0, w, HALO + n))
        n += w
    return pieces


def _chunk_of(l):
    return 0 if l < OWN_SPLIT[1] else (1 if l < OWN_SPLIT[2] else 2)


def _gout_spans():
    """Unpack spans per global tile t: list of (chunk, gout row0, p0, p1).

    gout_c rows are (core j, local l in chunk c) -> j*ow_c + (l - start_c).
    Core-independent by construction.
    """
    bounds = []
    for j in range(CORES):
        for c in range(NCH):
            bounds.append(j * ROWS + OWN_SPLIT[c])
    bounds.append(NUM_NODES)
    spans = {}
    for t in range(KT):
        g0, g1 = t * KP, min((t + 1) * KP, NUM_NODES)
        spans[t] = []
        g = g0
        while g < g1:
            j, l = g // ROWS, g % ROWS
            c = _chunk_of(l)
            ow = OWN_SPLIT[c + 1] - OWN_SPLIT[c]
            row0 = j * ow + (l - OWN_SPLIT[c])
            nxt = min(b for b in bounds if b > g)
            ge = min(g1, nxt)
            spans[t].append((c, row0, g - g0, ge - g0))
            g = ge
    return spans


def _tile_classes(spans):
    """Partition tiles by the latest gather they depend on."""
    cls = [[], [], []]
    for t in range(KT):
        mx = max(c for (c, _, _, _) in spans[t])
        cls[mx].append(t)
    return cls


def build_program():
    import concourse.bass as bass
    import concourse.mybir as mybir
    import concourse.tile as tile
    from concourse import bacc

    f16 = mybir.dt.float16
    f32 = mybir.dt.float32
    AF = mybir.ActivationFunctionType
    ALU = mybir.AluOpType

    nc = bacc.Bacc("TRN2", num_devices=CORES)

    GW = GSZ * CW                   # group width in L-cols (8 tiles)
    OWS = [OWN_SPLIT[c + 1] - OWN_SPLIT[c] for c in range(NCH)]

    # ---- inputs ----
    a2t_d = nc.dram_tensor("a2t", [NCH * NGRP * KP, GW], f16,
                           kind="ExternalInput")
    x_d = nc.dram_tensor("xn", [KP, KT * IN_FEAT], f16, kind="ExternalInput")
    w1_d = nc.dram_tensor("w1", [IN_FEAT, HID], f16, kind="ExternalInput")
    w2_d = nc.dram_tensor("w2", [HID, HID], f16, kind="ExternalInput")
    wiht_d = nc.dram_tensor("wiht", [HID, 3 * HID], f16, kind="ExternalInput")
    whht_d = nc.dram_tensor("whht", [HID, 3 * HID], f16, kind="ExternalInput")
    fcwt_d = nc.dram_tensor("fcwt", [HID, OUT], f16, kind="ExternalInput")
    ident_d = nc.dram_tensor("ident", [KP, KP], f16, kind="ExternalInput")
    b1c_d = nc.dram_tensor("b1c", [KP, 2], f32, kind="ExternalInput")
    b2c_d = nc.dram_tensor("b2c", [KP, 2], f32, kind="ExternalInput")
    gib_d = nc.dram_tensor("gib", [KP, 6], f32, kind="ExternalInput")
    bhn_d = nc.dram_tensor("bhn", [KP, 2], f32, kind="ExternalInput")
    fcb_d = nc.dram_tensor("fcb", [KP, 1], f32, kind="ExternalInput")
    patch_d = nc.dram_tensor("patch", [KP, 12], f32, kind="ExternalInput")
    out_d = nc.dram_tensor("out_t", [OUT, ROWS], f32, kind="ExternalOutput")

    spans = _gout_spans()
    cls = _tile_classes(spans)
    # GCN2 chunk-0 K-consumption order: by gather arrival
    c0_order = cls[0] + cls[1] + cls[2]

    with tile.TileContext(nc) as tc:
        with (
            tc.tile_pool(name="const", bufs=1) as cpool,
            tc.tile_pool(name="big", bufs=1) as big,
            tc.tile_pool(name="tmp", bufs=3) as tpool,
            tc.tile_pool(name="dram", bufs=1, space="DRAM") as dpool,
        ):
            psxw_cm = tc.tile_pool(name="psxw", bufs=2, space="PSUM")
            psxw = psxw_cm.__enter__()
            # ---- constants ----
            x_sb = cpool.tile([KP, KT, IN_FEAT], f16)
            w1_sb = cpool.tile([IN_FEAT, HID], f16)
            w2_sb = cpool.tile([KP, 2, HID], f16)
            wiht_sb = cpool.tile([KP, 2, 3 * HID], f16)
            whht_sb = cpool.tile([KP, 2, 3 * HID], f16)
            fcwt_sb = cpool.tile([KP, 2, OUT], f16)
            ident_sb = cpool.tile([KP, KP], f16)
            b1c_sb = cpool.tile([KP, 2], f32)
            b2c_sb = cpool.tile([KP, 2], f32)
            gib_sb = cpool.tile([KP, 6], f32)
            bhn_sb = cpool.tile([KP, 2], f32)
            fcb_sb = cpool.tile([KP, 1], f32)
            patch_sb = cpool.tile([KP, 12], f32)

            nc.sync.dma_start(ident_sb[:], ident_d[:])
            # tiny AllGather issued FIRST: the one-time ncfw collective setup
            # (~60us) runs behind GCN1's stream, not in front of gather 0
            ccw_in = dpool.tile([CORES, 64], f16)
            ccw_out = dpool.tile([CORES * CORES, 64], f16, addr_space="Shared")
            nc.sync.dma_start(ccw_in[0:8, :], ident_sb[0:8, 0:64])
            nc.gpsimd.collective_compute(
                "AllGather", mybir.AluOpType.bypass,
                replica_groups=[list(range(CORES))],
                ins=[ccw_in.opt()], outs=[ccw_out.opt()])
            hx = (KT // 2) * IN_FEAT
            nc.sync.dma_start(x_sb[:, :KT // 2, :], x_d[:, :hx])
            nc.scalar.dma_start(x_sb[:, KT // 2:, :], x_d[:, hx:])
            nc.sync.dma_start(w1_sb[:], w1_d[:])
            nc.scalar.dma_start(b1c_sb[:], b1c_d[:])
            for k in range(2):
                nc.sync.dma_start(w2_sb[:, k, :], w2_d[k * KP:(k + 1) * KP, :])
            nc.scalar.dma_start(b2c_sb[:], b2c_d[:])

            # ACT table pre-warm (sigmoid/tanh set) + PE HAM warm-up burst
            preh = cpool.tile([1, 16], f16)
            nc.scalar.activation(preh[0:1, 0:8], ident_sb[0:1, 0:8],
                                 AF.Sigmoid)
            nc.scalar.activation(preh[0:1, 8:16], ident_sb[0:1, 0:8], AF.Tanh)
            for i in range(40):
                psd = psxw.tile([KP, 512], f32, tag="xwps", name=f"warm_{i}")
                nc.tensor.matmul(psd[:, :KP], ident_sb[:], ident_sb[:],
                                 start=True, stop=True)
            psxw_cm.__exit__(None, None, None)

            # PSUM plan: p1 (psA/psH/psW) covers GCN1; after it closes, a
            # single 8-bank pool with rotating tags covers GCN2/GI/sweeps/FC.
            p1_cm = tc.tile_pool(name="p1ps", bufs=1, space="PSUM")
            p1 = p1_cm.__enter__()

            aret_cm = tc.tile_pool(name="aret", bufs=1)
            aret = aret_cm.__enter__()
            aret_t = aret.tile([KP, NGRP, GW], f16)
            ap_cm = tc.tile_pool(name="astream", bufs=2)
            apool = ap_cm.__enter__()

            h1t_sb = big.tile([KP, 2, L], f16)
            xw2g_sb = big.tile([KP, KT, HID], f16)
            h2t_sb = big.tile([KP, 2, L], f16)
            gi_sb = big.tile([KP, 6, L], f16)
            nc.vector.memset(xw2g_sb[:, KT - 1, :], 0.0)
            bounce = [dpool.tile([OWS[c], HID], f16, name=f"bnc_{c}")
                      for c in range(NCH)]
            gout = [dpool.tile([CORES * OWS[c], HID], f16, addr_space="Shared",
                               name=f"gout_{c}")
                    for c in range(NCH)]

            dma_engs = [nc.sync, nc.scalar]

            def stream_group(c, g, di):
                """DMA one A group; returns the SBUF tile holding it."""
                if c == 0:
                    at = aret_t[:, g, :]
                else:
                    att = apool.tile([KP, GW], f16, tag="a")
                    at = att
                dma_engs[di % 2].dma_start(
                    at, a2t_d[(c * NGRP + g) * KP:(c * NGRP + g + 1) * KP, :])
                return at


            def unpack_tiles(tiles):
                for i, t in enumerate(tiles):
                    for si, (c, row0, p0, p1_) in enumerate(spans[t]):
                        eng = dma_engs[(i + si) % 2]
                        eng.dma_start(xw2g_sb[p0:p1_, t, :],
                                      gout[c][row0:row0 + (p1_ - p0), :])

            def gcn1_chunk(c):
                """Stream chunk c, GCN1, xw2 pieces, bounce+gather."""
                psA = p1.tile([KP, 512], f32, tag="psA", name=f"psA_{c}")
                for g in range(NGRP):
                    at = stream_group(c, g, g)
                    for j in range(GSZ):
                        k = GSZ * g + j
                        if k >= KT:
                            break
                        nc.tensor.matmul(
                            psA[:IN_FEAT, :CW], x_sb[:, k, :],
                            at[:, j * CW:(j + 1) * CW],
                            start=(k == 0), stop=(k == KT - 1))
                agg_sb = tpool.tile([IN_FEAT, CW], f16, tag="agg")
                nc.vector.tensor_copy(agg_sb[:, :], psA[:IN_FEAT, :CW])
                for mm in range(2):
                    psH = p1.tile([KP, 512], f32, tag="psH",
                                  name=f"psH{mm}_{c}")
                    nc.tensor.matmul(psH[:, :CW],
                                     w1_sb[:, mm * KP:(mm + 1) * KP],
                                     agg_sb[:, :], start=True, stop=True)
                    nc.scalar.activation(h1t_sb[:, mm, c * CW:(c + 1) * CW],
                                         psH[:, :CW], AF.Relu,
                                         bias=b1c_sb[:, mm:mm + 1])
                # xw2 for this chunk's own rows -> bounce DRAM -> AllGather
                for pi, (roff, pw, lc0) in enumerate(_own_pieces(c)):
                    psW = p1.tile([KP, 512], f32, tag="psW",
                                  name=f"psW_{c}_{pi}")
                    for kf in range(2):
                        nc.tensor.matmul(
                            psW[:pw, :HID],
                            h1t_sb[:, kf, lc0:lc0 + pw],
                            w2_sb[:, kf, :], start=(kf == 0), stop=(kf == 1))
                    pc_sb = tpool.tile([KP, HID], f16, tag="pc")
                    nc.vector.tensor_copy(pc_sb[:pw, :], psW[:pw, :HID])
                    nc.sync.dma_start(bounce[c][roff:roff + pw, :],
                                      pc_sb[:pw, :])
                nc.gpsimd.collective_compute(
                    "AllGather", mybir.AluOpType.bypass,
                    replica_groups=[list(range(CORES))],
                    ins=[bounce[c].opt()], outs=[gout[c].opt()])
                unpack_tiles(cls[c])

            # ---- GCN1 over the 3 chunks (stream order 0,1,2) ----
            gcn1_chunk(0)
            for k in range(2):
                nc.sync.dma_start(wiht_sb[:, k, :],
                                  wiht_d[k * KP:(k + 1) * KP, :])
                nc.scalar.dma_start(whht_sb[:, k, :],
                                    whht_d[k * KP:(k + 1) * KP, :])
                nc.sync.dma_start(fcwt_sb[:, k, :],
                                  fcwt_d[k * KP:(k + 1) * KP, :])
            nc.scalar.dma_start(gib_sb[:], gib_d[:])
            nc.sync.dma_start(bhn_sb[:], bhn_d[:])
            nc.scalar.dma_start(fcb_sb[:], fcb_d[:])
            nc.sync.dma_start(patch_sb[:], patch_d[:])
            gcn1_chunk(1)
            gcn1_chunk(2)

            p1_cm.__exit__(None, None, None)
            pgi_cm = tc.tile_pool(name="pgi", bufs=1, space="PSUM")
            pgi = pgi_cm.__enter__()
            pscnt = [0]

            def pstile(name):
                t = pgi.tile([KP, 512], f32, tag=f"p{pscnt[0] % 8}", name=name)
                pscnt[0] += 1
                return t

            # ---- GCN2 c0 (retained A, K in gather-arrival order), c1/c2 ----
            def gi_chunk(c):
                for m in range(6):
                    psg = pstile(f"psgi_{c}_{m}")
                    for kf in range(2):
                        nc.tensor.matmul(psg[:, :CW],
                                         wiht_sb[:, kf, m * KP:(m + 1) * KP],
                                         h2t_sb[:, kf, c * CW:(c + 1) * CW],
                                         start=(kf == 0), stop=(kf == 1))
                    nc.vector.tensor_scalar_add(
                        gi_sb[:, m, c * CW:(c + 1) * CW],
                        psg[:, :CW], gib_sb[:, m:m + 1])

            def h2_chunk(c, psv):
                for mm in range(2):
                    nc.scalar.activation(h2t_sb[:, mm, c * CW:(c + 1) * CW],
                                         psv[mm][:, :CW], AF.Relu,
                                         bias=b2c_sb[:, mm:mm + 1])

            ps2 = [pstile(f"ps2_c0_{mm}") for mm in range(2)]
            for i, k in enumerate(c0_order):
                for mm in range(2):
                    nc.tensor.matmul(
                        ps2[mm][:, :CW],
                        xw2g_sb[:, k, mm * KP:(mm + 1) * KP],
                        aret_t[:, k // GSZ, (k % GSZ) * CW:(k % GSZ + 1) * CW],
                        start=(i == 0), stop=(i == KT - 1))
            h2_chunk(0, ps2)
            gi_chunk(0)
            for c in (1, 2):
                psv = [pstile(f"ps2_c{c}_{mm}") for mm in range(2)]
                for g in range(NGRP):
                    at = stream_group(c, g, g)
                    for j in range(GSZ):
                        k = GSZ * g + j
                        if k >= KT:
                            break
                        for mm in range(2):
                            nc.tensor.matmul(
                                psv[mm][:, :CW],
                                xw2g_sb[:, k, mm * KP:(mm + 1) * KP],
                                at[:, j * CW:(j + 1) * CW],
                                start=(k == 0), stop=(k == KT - 1))
                h2_chunk(c, psv)
                gi_chunk(c)
            # per-core GI patch on the first HALO columns
            for m in range(6):
                nc.vector.tensor_scalar(gi_sb[:, m, :HALO], gi_sb[:, m, :HALO],
                                        patch_sb[:, m:m + 1],
                                        patch_sb[:, 6 + m:7 + m],
                                        ALU.mult, ALU.add)

            ap_cm.__exit__(None, None, None)
            aret_cm.__exit__(None, None, None)

            # ---- GRU fixed-point sweeps ----
            # hsh[p][:, mm, 1+t] = h_t of sweep s with p = s%2; col 0 = 0.
            hsh = [big.tile([KP, 2, L + 1], f16, name=f"hsh{p}")
                   for p in range(2)]
            for p in range(2):
                for mm in range(2):
                    nc.vector.memset(hsh[p][:, mm, 0:1], 0.0)

            for s in range(SWEEPS):
                cur = hsh[s % 2]
                prv = hsh[(s - 1) % 2]
                z_sb = big.tile([KP, 2, L], f16, tag="Z")
                b_sb = big.tile([KP, 2, L], f16, tag="B")
                for c in range(NCH):
                    c0, c1 = c * CW, (c + 1) * CW
                    if s == 0:
                        # h = 0: gates direct from gi, no matmuls; keep the
                        # PE warm with throwaway matmuls meanwhile.
                        psd = pstile(f"kw_{s}_{c}")
                        nc.tensor.matmul(psd[:, :CW], ident_sb[:],
                                         gi_sb[:, c % 6, c0:c1],
                                         start=True, stop=True)
                        for mm in range(2):
                            r_t = tpool.tile([KP, CW], f16, tag="r")
                            t_t = tpool.tile([KP, CW], f16, tag="t")
                            un_t = tpool.tile([KP, CW], f16, tag="un")
                            n_t = tpool.tile([KP, CW], f16, tag="n")
                            nc.scalar.activation(r_t[:, :],
                                                 gi_sb[:, mm, c0:c1],
                                                 AF.Sigmoid)
                            nc.scalar.activation(z_sb[:, mm, c0:c1],
                                                 gi_sb[:, 2 + mm, c0:c1],
                                                 AF.Sigmoid)
                            nc.vector.tensor_scalar_mul(t_t[:, :], r_t[:, :],
                                                        bhn_sb[:, mm:mm + 1])
                            nc.vector.tensor_add(un_t[:, :], t_t[:, :],
                                                 gi_sb[:, 4 + mm, c0:c1])
                            nc.scalar.activation(n_t[:, :], un_t[:, :],
                                                 AF.Tanh)
                            nc.vector.scalar_tensor_tensor(
                                b_sb[:, mm, c0:c1], z_sb[:, mm, c0:c1], 1.0,
                                n_t[:, :], ALU.subtract, ALU.mult)
                            nc.vector.tensor_tensor_scan(
                                cur[:, mm, 1 + c0:1 + c1],
                                z_sb[:, mm, c0:c1], b_sb[:, mm, c0:c1],
                                cur[:, mm, c0:c0 + 1],
                                ALU.mult, ALU.subtract)
                        continue
                    psg = [pstile(f"psu_{s}_{c}_{m}") for m in range(6)]
                    for m in range(4):
                        nc.tensor.matmul(psg[m][:, :CW], ident_sb[:],
                                         gi_sb[:, m, c0:c1],
                                         start=True, stop=False)
                    for m in range(6):
                        for kf in range(2):
                            nc.tensor.matmul(
                                psg[m][:, :CW],
                                whht_sb[:, kf, m * KP:(m + 1) * KP],
                                prv[:, kf, c0:c1],
                                start=(m >= 4 and kf == 0), stop=(kf == 1))
                    for mm in range(2):
                        r_t = tpool.tile([KP, CW], f16, tag="r")
                        t_t = tpool.tile([KP, CW], f16, tag="t")
                        un_t = tpool.tile([KP, CW], f16, tag="un")
                        n_t = tpool.tile([KP, CW], f16, tag="n")
                        nc.scalar.activation(r_t[:, :], psg[mm][:, :CW],
                                             AF.Sigmoid)
                        nc.scalar.activation(z_sb[:, mm, c0:c1],
                                             psg[2 + mm][:, :CW], AF.Sigmoid)
                        nc.vector.scalar_tensor_tensor(
                            t_t[:, :], psg[4 + mm][:, :CW],
                            bhn_sb[:, mm:mm + 1], r_t[:, :],
                            ALU.add, ALU.mult)
                        nc.vector.tensor_add(un_t[:, :], t_t[:, :],
                                             gi_sb[:, 4 + mm, c0:c1])
                        nc.scalar.activation(n_t[:, :], un_t[:, :], AF.Tanh)
                        nc.vector.scalar_tensor_tensor(
                            b_sb[:, mm, c0:c1], z_sb[:, mm, c0:c1], 1.0,
                            n_t[:, :], ALU.subtract, ALU.mult)
                        # exact chunk scan with per-partition carry
                        nc.vector.tensor_tensor_scan(
                            cur[:, mm, 1 + c0:1 + c1],
                            z_sb[:, mm, c0:c1], b_sb[:, mm, c0:c1],
                            cur[:, mm, c0:c0 + 1],
                            ALU.mult, ALU.subtract)

            # ---- final Linear on the real rows (skip halo) ----
            hfin = hsh[(SWEEPS - 1) % 2]
            out_sb = cpool.tile([4, ROWS], f32)
            for fi, (c0, c1) in enumerate(_chunks(ROWS)):
                cw = c1 - c0
                psf = pstile(f"psf_{fi}")
                for kf in range(2):
                    nc.tensor.matmul(psf[:OUT, :cw], fcwt_sb[:, kf, :],
                                     hfin[:, kf, HALO + 1 + c0:HALO + 1 + c1],
                                     start=(kf == 0), stop=(kf == 1))
                nc.vector.tensor_scalar_add(out_sb[:OUT, c0:c1],
                                            psf[:OUT, :cw], fcb_sb[:OUT, :])
            nc.sync.dma_start(out_d[:], out_sb[:OUT, :])

            pgi_cm.__exit__(None, None, None)

    nc.compile()
    return nc


def host_prepare(inputs):
    """Build the per-core input maps from the full problem inputs."""
    x = np.asarray(inputs["x"], np.float32)
    ei = np.asarray(inputs["edge_index"])
    W1 = np.asarray(inputs["W1"], np.float32)
    b1 = np.asarray(inputs["b1"], np.float32)
    W2 = np.asarray(inputs["W2"], np.float32)
    b2 = np.asarray(inputs["b2"], np.float32)
    W_ih = np.asarray(inputs["W_ih"], np.float32)
    W_hh = np.asarray(inputs["W_hh"], np.float32)
    b_ih = np.asarray(inputs["b_ih"], np.float32)
    b_hh = np.asarray(inputs["b_hh"], np.float32)
    fc_w = np.asarray(inputs["fc_w"], np.float32)
    fc_b = np.asarray(inputs["fc_b"], np.float32)

    N = NUM_NODES
    src, dst = ei[0].astype(np.int64), ei[1].astype(np.int64)
    deg = np.bincount(dst, minlength=N).astype(np.float64) + 1.0
    dinv = 1.0 / np.sqrt(deg)
    # A_T[s, d] = normalization weight of edge s->d (plus self loops)
    at = np.zeros((N, N), np.float32)
    np.add.at(at, (src, dst), (dinv[src] * dinv[dst]).astype(np.float32))
    idx = np.arange(N)
    at[idx, idx] += (dinv * dinv).astype(np.float32)
    at16 = at.astype(np.float16)
    del at

    # x natural layout [128, KT*64]
    xpad = np.zeros((NKPAD * KP, IN_FEAT), np.float16)
    xpad[:N] = x.astype(np.float16)
    xn = np.ascontiguousarray(
        xpad[:KT * KP].reshape(KT, KP, IN_FEAT).transpose(1, 0, 2)
    ).reshape(KP, KT * IN_FEAT)

    common = {
        "xn": xn,
        "w1": W1.astype(np.float16),
        "w2": W2.astype(np.float16),
        "wiht": W_ih.T.astype(np.float16),
        "whht": W_hh.T.astype(np.float16),
        "fcwt": fc_w.T.astype(np.float16),
        "ident": np.eye(KP, dtype=np.float16),
        "b1c": b1.reshape(2, KP).T.astype(np.float32).copy(),
        "b2c": b2.reshape(2, KP).T.astype(np.float32).copy(),
        "gib": (b_ih + np.concatenate([b_hh[:2 * HID],
                                       np.zeros(HID, np.float32)])
                ).reshape(6, KP).T.astype(np.float32).copy(),
        "bhn": b_hh[2 * HID:].reshape(2, KP).T.astype(np.float32).copy(),
        "fcb": np.concatenate([fc_b, np.zeros(KP - OUT, np.float32)]
                              ).reshape(KP, 1),
    }

    in_maps = []
    for c in range(CORES):
        r0, r1 = c * ROWS, (c + 1) * ROWS
        strip = np.zeros((NKPAD * KP, L), np.float16)
        if c == 0:
            strip[:N, HALO:] = at16[:, r0:r1]
        else:
            strip[:N, :] = at16[:, r0 - HALO:r1]
        # chunk-major interleave: [(c*NGRP+g)*128+p, j*CW+cc]
        a2t = np.ascontiguousarray(
            strip.reshape(NGRP, GSZ, KP, NCH, CW).transpose(3, 0, 2, 1, 4)
        ).reshape(NCH * NGRP * KP, GSZ * CW)
        patch = np.zeros((KP, 12), np.float32)
        if c == 0:
            # mul=0; add=-60 for r,z gate tiles, 0 for n tiles -> pad cols
            # produce exactly h=0 so row 0 starts from the true h0=0.
            patch[:, 6:10] = -60.0
        else:
            patch[:, 0:6] = 1.0
        in_maps.append({**common, "a2t": a2t, "patch": patch})
    return in_maps


def assemble_output(results):
    outs = [r["out_t"].T for r in results]          # each [ROWS, OUT]
    full = np.concatenate(outs, axis=0).astype(np.float32)
    return full[None]                               # [1, N, OUT]


def kernel(**inputs) -> np.ndarray:
    from concourse import bass_utils

    if "nc" not in _CACHE:
        _CACHE["nc"] = build_program()
    nc = _CACHE["nc"]
    in_maps = host_prepare(inputs)
    res = bass_utils.run_bass_kernel_spmd(
        nc, in_maps, core_ids=list(range(CORES)))
    return assemble_output(res.results)


if __name__ == "__main__":
    import reference

    inputs = {k: np.asarray(v) for k, v in reference.setup_inputs().items()}
    out = kernel(**inputs)
    print("kernel out", out.shape, out.dtype)
    np.save("/root/problem/kernel_out.npy", out)


# revision 30
# speedup vs baseline: 1.1248x; 1.1248x over previous
"""DCRNN (2x GCNConv + GRU-over-nodes + Linear) on 8 Trainium2 cores, v3.

Strategy
--------
* Dense normalized adjacency (A+I, D^-1/2 scaled) sharded column-wise
  (1250 own cols + 64-col left halo per core), fp16, laid out
  column-chunk-major (3 chunks x 79 K-tiles, streamed in 8-tile groups
  with 7KB descriptor rows) so each layer's output chunk finishes as
  soon as its part of the stream has been consumed.
* GCN1 uses associativity: h1 = relu((A^T_strip.T @ x) @ W1 + b1) -- the
  big A matmul runs at M=64 (one matmul per K-tile) and there is no
  separate x@W1 pass.
* Per GCN1 chunk: xw2 = h1@W2 for the chunk's own rows, bounced to DRAM
  and AllGathered (3 chunked collectives pipelined behind the A stream).
  GCN2 consumes the gathered xw2 as stationary operands, chunk-major.
  Chunk 0 is streamed first and retained in SBUF; GCN2-chunk-0 matmuls
  are ordered by gather arrival and interleaved into GCN1's DMA-paced
  windows, so by the time the stream ends most of GCN2-c0 is done.
* GRU over the node sequence: 8 Jacobi fixed-point sweeps; gates from the
  previous sweep's h (double-buffered hidden state so the exact affine
  scan -- tensor_tensor_scan, chunked with per-partition carries --
  pipelines against the next chunk's matmuls); sweep 0 is matmul-free.
* Final Linear on the own rows; host concatenates the 8 shards.
"""

import numpy as np

NUM_NODES = 10000
IN_FEAT = 64
HID = 256
OUT = 3
CORES = 8
ROWS = NUM_NODES // CORES          # 1250
HALO = 64
L = ROWS + HALO                    # 1314 local sequence length
CW = L // 3                        # 438 chunk width
NCH = 3
KP = 128
KT = (NUM_NODES + KP - 1) // KP    # 79 K-tiles (global node tiles)
NKPAD = 80                         # K-tiles padded to a multiple of GSZ
GSZ = 5                            # K-tiles per stream group
NGRP = NKPAD // GSZ                # 10 stream groups per chunk
SWEEPS = 8
# own-node ranges per chunk (L-cols [0,64) are halo)
OWN_SPLIT = [0, CW - HALO, 2 * CW - HALO, ROWS]     # [0, 374, 812, 1250]

_CACHE = {}


def _chunks(total, step=512):
    return [(c, min(c + step, total)) for c in range(0, total, step)]


def _own_pieces(c):
    """<=128-wide pieces of chunk c's own-node range: (rowoff, width, lcol0).

    rowoff is relative to the chunk's bounce buffer; lcol0 indexes h1t's
    L axis (own node n lives at L-col HALO+n).
    """
    n0, n1 = OWN_SPLIT[c], OWN_SPLIT[c + 1]
    pieces = []
    n = n0
    while n < n1:
        w = min(KP, n1 - n)
        pieces.append((n - n0, w, HALO + n))
        n += w
    return pieces


def _chunk_of(l):
    return 0 if l < OWN_SPLIT[1] else (1 if l < OWN_SPLIT[2] else 2)


GSPLIT = OWN_SPLIT[2]               # 812: gather A = own rows [0,812)
GA_W, GB_W = GSPLIT, ROWS - GSPLIT  # 812 / 438 rows per core


def _gout_spans():
    """Unpack spans per global tile t: list of (buf, gout row0, p0, p1).

    gout_A rows: (core j, l<812) -> j*812 + l.
    gout_B rows: (core j, l>=812) -> j*438 + (l-812).
    Core-independent by construction.
    """
    bounds = []
    for j in range(CORES):
        bounds += [j * ROWS, j * ROWS + GSPLIT]
    bounds.append(NUM_NODES)
    spans = {}
    for t in range(KT):
        g0, g1 = t * KP, min((t + 1) * KP, NUM_NODES)
        spans[t] = []
        g = g0
        while g < g1:
            j, l = g // ROWS, g % ROWS
            if l < GSPLIT:
                buf, row0 = 0, j * GA_W + l
            else:
                buf, row0 = 1, j * GB_W + (l - GSPLIT)
            nxt = min(b for b in bounds if b > g)
            ge = min(g1, nxt)
            spans[t].append((buf, row0, g - g0, ge - g0))
            g = ge
    return spans


def _tile_classes(spans):
    """Partition tiles by the latest gather they depend on."""
    cls = [[], []]
    for t in range(KT):
        mx = max(b for (b, _, _, _) in spans[t])
        cls[mx].append(t)
    return cls


def build_program():
    import concourse.bass as bass
    import concourse.mybir as mybir
    import concourse.tile as tile
    from concourse import bacc

    f16 = mybir.dt.float16
    f32 = mybir.dt.float32
    AF = mybir.ActivationFunctionType
    ALU = mybir.AluOpType

    nc = bacc.Bacc("TRN2", num_devices=CORES)

    GW = GSZ * CW                   # group width in L-cols (8 tiles)
    OWS = [OWN_SPLIT[c + 1] - OWN_SPLIT[c] for c in range(NCH)]

    # ---- inputs ----
    a2t_d = nc.dram_tensor("a2t", [NCH * NGRP * KP, GW], f16,
                           kind="ExternalInput")
    x_d = nc.dram_tensor("xn", [KP, KT * IN_FEAT], f16, kind="ExternalInput")
    w1_d = nc.dram_tensor("w1", [IN_FEAT, HID], f16, kind="ExternalInput")
    w2_d = nc.dram_tensor("w2", [HID, HID], f16, kind="ExternalInput")
    wiht_d = nc.dram_tensor("wiht", [HID, 3 * HID], f16, kind="ExternalInput")
    whht_d = nc.dram_tensor("whht", [HID, 3 * HID], f16, kind="ExternalInput")
    fcwt_d = nc.dram_tensor("fcwt", [HID, OUT], f16, kind="ExternalInput")
    ident_d = nc.dram_tensor("ident", [KP, KP], f16, kind="ExternalInput")
    b1c_d = nc.dram_tensor("b1c", [KP, 2], f32, kind="ExternalInput")
    b2c_d = nc.dram_tensor("b2c", [KP, 2], f32, kind="ExternalInput")
    gib_d = nc.dram_tensor("gib", [KP, 6], f32, kind="ExternalInput")
    bhn_d = nc.dram_tensor("bhn", [KP, 2], f32, kind="ExternalInput")
    fcb_d = nc.dram_tensor("fcb", [KP, 1], f32, kind="ExternalInput")
    patch_d = nc.dram_tensor("patch", [KP, 12], f32, kind="ExternalInput")
    out_d = nc.dram_tensor("out_t", [OUT, ROWS], f32, kind="ExternalOutput")

    spans = _gout_spans()
    cls = _tile_classes(spans)
    # GCN2 chunk-0 K-consumption order: by gather arrival
    c0_order = cls[0] + cls[1]

    with tile.TileContext(nc) as tc:
        with (
            tc.tile_pool(name="const", bufs=1) as cpool,
            tc.tile_pool(name="big", bufs=1) as big,
            tc.tile_pool(name="tmp", bufs=3) as tpool,
            tc.tile_pool(name="dram", bufs=1, space="DRAM") as dpool,
        ):
            psxw_cm = tc.tile_pool(name="psxw", bufs=2, space="PSUM")
            psxw = psxw_cm.__enter__()
            # ---- constants ----
            x_sb = cpool.tile([KP, KT, IN_FEAT], f16)
            w1_sb = cpool.tile([IN_FEAT, HID], f16)
            w2_sb = cpool.tile([KP, 2, HID], f16)
            wiht_sb = cpool.tile([KP, 2, 3 * HID], f16)
            whht_sb = cpool.tile([KP, 2, 3 * HID], f16)
            fcwt_sb = cpool.tile([KP, 2, OUT], f16)
            ident_sb = cpool.tile([KP, KP], f16)
            b1c_sb = cpool.tile([KP, 2], f32)
            b2c_sb = cpool.tile([KP, 2], f32)
            gib_sb = cpool.tile([KP, 6], f32)
            bhn_sb = cpool.tile([KP, 2], f32)
            fcb_sb = cpool.tile([KP, 1], f32)
            patch_sb = cpool.tile([KP, 12], f32)

            nc.sync.dma_start(ident_sb[:], ident_d[:])
            # tiny AllGather issued FIRST: the one-time ncfw collective setup
            # (~60us) runs behind GCN1's stream, not in front of gather 0
            ccw_in = dpool.tile([CORES, 64], f16)
            ccw_out = dpool.tile([CORES * CORES, 64], f16, addr_space="Shared")
            nc.sync.dma_start(ccw_in[0:8, :], ident_sb[0:8, 0:64])
            nc.gpsimd.collective_compute(
                "AllGather", mybir.AluOpType.bypass,
                replica_groups=[list(range(CORES))],
                ins=[ccw_in.opt()], outs=[ccw_out.opt()])
            hx = (KT // 2) * IN_FEAT
            nc.sync.dma_start(x_sb[:, :KT // 2, :], x_d[:, :hx])
            nc.scalar.dma_start(x_sb[:, KT // 2:, :], x_d[:, hx:])
            nc.sync.dma_start(w1_sb[:], w1_d[:])
            nc.scalar.dma_start(b1c_sb[:], b1c_d[:])
            for k in range(2):
                nc.sync.dma_start(w2_sb[:, k, :], w2_d[k * KP:(k + 1) * KP, :])
            nc.scalar.dma_start(b2c_sb[:], b2c_d[:])

            # ACT table pre-warm (sigmoid/tanh set) + PE HAM warm-up burst
            preh = cpool.tile([1, 16], f16)
            nc.scalar.activation(preh[0:1, 0:8], ident_sb[0:1, 0:8],
                                 AF.Sigmoid)
            nc.scalar.activation(preh[0:1, 8:16], ident_sb[0:1, 0:8], AF.Tanh)
            for i in range(40):
                psd = psxw.tile([KP, 512], f32, tag="xwps", name=f"warm_{i}")
                nc.tensor.matmul(psd[:, :KP], ident_sb[:], ident_sb[:],
                                 start=True, stop=True)
            psxw_cm.__exit__(None, None, None)

            # PSUM plan: p1 (psA/psH/psW) covers GCN1; after it closes, a
            # single 8-bank pool with rotating tags covers GCN2/GI/sweeps/FC.
            p1_cm = tc.tile_pool(name="p1ps", bufs=1, space="PSUM")
            p1 = p1_cm.__enter__()

            aret_cm = tc.tile_pool(name="aret", bufs=1)
            aret = aret_cm.__enter__()
            aret_t = aret.tile([KP, NGRP, GW], f16)
            ap_cm = tc.tile_pool(name="astream", bufs=3)
            apool = ap_cm.__enter__()

            h1t_sb = big.tile([KP, 2, L], f16)
            xw2g_sb = big.tile([KP, KT, HID], f16)
            h2t_sb = big.tile([KP, 2, L], f16)
            gi_sb = big.tile([KP, 6, L], f16)
            nc.vector.memset(xw2g_sb[:, KT - 1, :], 0.0)
            bounce = [dpool.tile([GA_W, HID], f16, name="bnc_A"),
                      dpool.tile([GB_W, HID], f16, name="bnc_B")]
            gout = [dpool.tile([CORES * GA_W, HID], f16, addr_space="Shared",
                               name="gout_A"),
                    dpool.tile([CORES * GB_W, HID], f16, addr_space="Shared",
                               name="gout_B")]

            dma_engs = [nc.sync, nc.scalar]

            def stream_group(c, g, di):
                """DMA one A group; returns the SBUF tile holding it."""
                if c == 0:
                    at = aret_t[:, g, :]
                else:
                    att = apool.tile([KP, GW], f16, tag="a")
                    at = att
                dma_engs[di % 2].dma_start(
                    at, a2t_d[(c * NGRP + g) * KP:(c * NGRP + g + 1) * KP, :])
                return at


            def unpack_tiles(tiles):
                for i, t in enumerate(tiles):
                    for si, (c, row0, p0, p1_) in enumerate(spans[t]):
                        eng = dma_engs[(i + si) % 2]
                        eng.dma_start(xw2g_sb[p0:p1_, t, :],
                                      gout[c][row0:row0 + (p1_ - p0), :])

            def gcn1_chunk(c):
                """Stream chunk c, GCN1, xw2 pieces, bounce+gather."""
                psA = p1.tile([KP, 512], f32, tag="psA", name=f"psA_{c}")
                for g in range(NGRP):
                    at = stream_group(c, g, g)
                    for j in range(GSZ):
                        k = GSZ * g + j
                        if k >= KT:
                            break
                        nc.tensor.matmul(
                            psA[:IN_FEAT, :CW], x_sb[:, k, :],
                            at[:, j * CW:(j + 1) * CW],
                            start=(k == 0), stop=(k == KT - 1))
                agg_sb = tpool.tile([IN_FEAT, CW], f16, tag="agg")
                nc.vector.tensor_copy(agg_sb[:, :], psA[:IN_FEAT, :CW])
                for mm in range(2):
                    psH = p1.tile([KP, 512], f32, tag="psH",
                                  name=f"psH{mm}_{c}")
                    nc.tensor.matmul(psH[:, :CW],
                                     w1_sb[:, mm * KP:(mm + 1) * KP],
                                     agg_sb[:, :], start=True, stop=True)
                    nc.scalar.activation(h1t_sb[:, mm, c * CW:(c + 1) * CW],
                                         psH[:, :CW], AF.Relu,
                                         bias=b1c_sb[:, mm:mm + 1])
                # xw2 for this chunk's own rows -> bounce DRAM -> AllGather
                for pi, (roff, pw, lc0) in enumerate(_own_pieces(c)):
                    psW = p1.tile([KP, 512], f32, tag="psW",
                                  name=f"psW_{c}_{pi}")
                    for kf in range(2):
                        nc.tensor.matmul(
                            psW[:pw, :HID],
                            h1t_sb[:, kf, lc0:lc0 + pw],
                            w2_sb[:, kf, :], start=(kf == 0), stop=(kf == 1))
                    pc_sb = tpool.tile([KP, HID], f16, tag="pc")
                    nc.vector.tensor_copy(pc_sb[:pw, :], psW[:pw, :HID])
                    buf = 0 if c < 2 else 1
                    boff = OWN_SPLIT[c] if c < 2 else 0
                    nc.sync.dma_start(bounce[buf][boff + roff:
                                                  boff + roff + pw, :],
                                      pc_sb[:pw, :])

            # ---- GCN1 over the 3 chunks (stream order 0,1,2) ----
            gcn1_chunk(0)
            for k in range(2):
                nc.sync.dma_start(wiht_sb[:, k, :],
                                  wiht_d[k * KP:(k + 1) * KP, :])
                nc.scalar.dma_start(whht_sb[:, k, :],
                                    whht_d[k * KP:(k + 1) * KP, :])
                nc.sync.dma_start(fcwt_sb[:, k, :],
                                  fcwt_d[k * KP:(k + 1) * KP, :])
            nc.scalar.dma_start(gib_sb[:], gib_d[:])
            nc.sync.dma_start(bhn_sb[:], bhn_d[:])
            nc.scalar.dma_start(fcb_sb[:], fcb_d[:])
            nc.sync.dma_start(patch_sb[:], patch_d[:])
            gcn1_chunk(1)
            nc.gpsimd.collective_compute(
                "AllGather", mybir.AluOpType.bypass,
                replica_groups=[list(range(CORES))],
                ins=[bounce[0].opt()], outs=[gout[0].opt()])
            unpack_tiles(cls[0])
            gcn1_chunk(2)
            nc.gpsimd.collective_compute(
                "AllGather", mybir.AluOpType.bypass,
                replica_groups=[list(range(CORES))],
                ins=[bounce[1].opt()], outs=[gout[1].opt()])
            unpack_tiles(cls[1])

            p1_cm.__exit__(None, None, None)
            pgi_cm = tc.tile_pool(name="pgi", bufs=1, space="PSUM")
            pgi = pgi_cm.__enter__()
            pscnt = [0]

            def pstile(name):
                t = pgi.tile([KP, 512], f32, tag=f"p{pscnt[0] % 8}", name=name)
                pscnt[0] += 1
                return t

            # ---- GCN2 c0 (retained A, K in gather-arrival order), c1/c2 ----
            def gi_chunk(c):
                for m in range(6):
                    psg = pstile(f"psgi_{c}_{m}")
                    for kf in range(2):
                        nc.tensor.matmul(psg[:, :CW],
                                         wiht_sb[:, kf, m * KP:(m + 1) * KP],
                                         h2t_sb[:, kf, c * CW:(c + 1) * CW],
                                         start=(kf == 0), stop=(kf == 1))
                    nc.vector.tensor_scalar_add(
                        gi_sb[:, m, c * CW:(c + 1) * CW],
                        psg[:, :CW], gib_sb[:, m:m + 1])

            def h2_chunk(c, psv):
                for mm in range(2):
                    nc.scalar.activation(h2t_sb[:, mm, c * CW:(c + 1) * CW],
                                         psv[mm][:, :CW], AF.Relu,
                                         bias=b2c_sb[:, mm:mm + 1])

            ps2 = [pstile(f"ps2_c0_{mm}") for mm in range(2)]
            for i, k in enumerate(c0_order):
                for mm in range(2):
                    nc.tensor.matmul(
                        ps2[mm][:, :CW],
                        xw2g_sb[:, k, mm * KP:(mm + 1) * KP],
                        aret_t[:, k // GSZ, (k % GSZ) * CW:(k % GSZ + 1) * CW],
                        start=(i == 0), stop=(i == KT - 1))
            h2_chunk(0, ps2)
            gi_chunk(0)
            for c in (1, 2):
                psv = [pstile(f"ps2_c{c}_{mm}") for mm in range(2)]
                for g in range(NGRP):
                    at = stream_group(c, g, g)
                    for j in range(GSZ):
                        k = GSZ * g + j
                        if k >= KT:
                            break
                        for mm in range(2):
                            nc.tensor.matmul(
                                psv[mm][:, :CW],
                                xw2g_sb[:, k, mm * KP:(mm + 1) * KP],
                                at[:, j * CW:(j + 1) * CW],
                                start=(k == 0), stop=(k == KT - 1))
                h2_chunk(c, psv)
                gi_chunk(c)
            # per-core GI patch on the first HALO columns
            for m in range(6):
                nc.vector.tensor_scalar(gi_sb[:, m, :HALO], gi_sb[:, m, :HALO],
                                        patch_sb[:, m:m + 1],
                                        patch_sb[:, 6 + m:7 + m],
                                        ALU.mult, ALU.add)

            ap_cm.__exit__(None, None, None)
            aret_cm.__exit__(None, None, None)

            # ---- GRU fixed-point sweeps ----
            # hsh[p][:, mm, 1+t] = h_t of sweep s with p = s%2; col 0 = 0.
            hsh = [big.tile([KP, 2, L + 1], f16, name=f"hsh{p}")
                   for p in range(2)]
            for p in range(2):
                for mm in range(2):
                    nc.vector.memset(hsh[p][:, mm, 0:1], 0.0)

            for s in range(SWEEPS):
                cur = hsh[s % 2]
                prv = hsh[(s - 1) % 2]
                z_sb = big.tile([KP, 2, L], f16, tag="Z")
                b_sb = big.tile([KP, 2, L], f16, tag="B")
                for c in range(NCH):
                    c0, c1 = c * CW, (c + 1) * CW
                    if s == 0:
                        # h = 0: gates direct from gi, no matmuls; keep the
                        # PE warm with throwaway matmuls meanwhile.
                        psd = pstile(f"kw_{s}_{c}")
                        nc.tensor.matmul(psd[:, :CW], ident_sb[:],
                                         gi_sb[:, c % 6, c0:c1],
                                         start=True, stop=True)
                        for mm in range(2):
                            r_t = tpool.tile([KP, CW], f16, tag="r")
                            t_t = tpool.tile([KP, CW], f16, tag="t")
                            un_t = tpool.tile([KP, CW], f16, tag="un")
                            n_t = tpool.tile([KP, CW], f16, tag="n")
                            nc.scalar.activation(r_t[:, :],
                                                 gi_sb[:, mm, c0:c1],
                                                 AF.Sigmoid)
                            nc.scalar.activation(z_sb[:, mm, c0:c1],
                                                 gi_sb[:, 2 + mm, c0:c1],
                                                 AF.Sigmoid)
                            nc.vector.tensor_scalar_mul(t_t[:, :], r_t[:, :],
                                                        bhn_sb[:, mm:mm + 1])
                            nc.vector.tensor_add(un_t[:, :], t_t[:, :],
                                                 gi_sb[:, 4 + mm, c0:c1])
                            nc.scalar.activation(n_t[:, :], un_t[:, :],
                                                 AF.Tanh)
                            nc.vector.scalar_tensor_tensor(
                                b_sb[:, mm, c0:c1], z_sb[:, mm, c0:c1], 1.0,
                                n_t[:, :], ALU.subtract, ALU.mult)
                            nc.vector.tensor_tensor_scan(
                                cur[:, mm, 1 + c0:1 + c1],
                                z_sb[:, mm, c0:c1], b_sb[:, mm, c0:c1],
                                cur[:, mm, c0:c0 + 1],
                                ALU.mult, ALU.subtract)
                        continue
                    psg = [pstile(f"psu_{s}_{c}_{m}") for m in range(6)]
                    for m in range(4):
                        nc.tensor.matmul(psg[m][:, :CW], ident_sb[:],
                                         gi_sb[:, m, c0:c1],
                                         start=True, stop=False)
                    for m in range(6):
                        for kf in range(2):
                            nc.tensor.matmul(
                                psg[m][:, :CW],
                                whht_sb[:, kf, m * KP:(m + 1) * KP],
                                prv[:, kf, c0:c1],
                                start=(m >= 4 and kf == 0), stop=(kf == 1))
                    for mm in range(2):
                        r_t = tpool.tile([KP, CW], f16, tag="r")
                        t_t = tpool.tile([KP, CW], f16, tag="t")
                        un_t = tpool.tile([KP, CW], f16, tag="un")
                        n_t = tpool.tile([KP, CW], f16, tag="n")
                        nc.scalar.activation(r_t[:, :], psg[mm][:, :CW],
                                             AF.Sigmoid)
                        nc.scalar.activation(z_sb[:, mm, c0:c1],
                                             psg[2 + mm][:, :CW], AF.Sigmoid)
                        nc.vector.scalar_tensor_tensor(
                            t_t[:, :], psg[4 + mm][:, :CW],
                            bhn_sb[:, mm:mm + 1], r_t[:, :],
                            ALU.add, ALU.mult)
                        nc.vector.tensor_add(un_t[:, :], t_t[:, :],
                                             gi_sb[:, 4 + mm, c0:c1])
                        nc.scalar.activation(n_t[:, :], un_t[:, :], AF.Tanh)
                        nc.vector.scalar_tensor_tensor(
                            b_sb[:, mm, c0:c1], z_sb[:, mm, c0:c1], 1.0,
                            n_t[:, :], ALU.subtract, ALU.mult)
                        # exact chunk scan with per-partition carry
                        nc.vector.tensor_tensor_scan(
                            cur[:, mm, 1 + c0:1 + c1],
                            z_sb[:, mm, c0:c1], b_sb[:, mm, c0:c1],
                            cur[:, mm, c0:c0 + 1],
                            ALU.mult, ALU.subtract)

            # ---- final Linear on the real rows (skip halo) ----
            hfin = hsh[(SWEEPS - 1) % 2]
            out_sb = cpool.tile([4, ROWS], f32)
            for fi, (c0, c1) in enumerate(_chunks(ROWS)):
                cw = c1 - c0
                psf = pstile(f"psf_{fi}")
                for kf in range(2):
                    nc.tensor.matmul(psf[:OUT, :cw], fcwt_sb[:, kf, :],
                                     hfin[:, kf, HALO + 1 + c0:HALO + 1 + c1],
                                     start=(kf == 0), stop=(kf == 1))
                nc.vector.tensor_scalar_add(out_sb[:OUT, c0:c1],
                                            psf[:OUT, :cw], fcb_sb[:OUT, :])
            nc.sync.dma_start(out_d[:], out_sb[:OUT, :])

            pgi_cm.__exit__(None, None, None)

    nc.compile()
    return nc


def host_prepare(inputs):
    """Build the per-core input maps from the full problem inputs."""
    x = np.asarray(inputs["x"], np.float32)
    ei = np.asarray(inputs["edge_index"])
    W1 = np.asarray(inputs["W1"], np.float32)
    b1 = np.asarray(inputs["b1"], np.float32)
    W2 = np.asarray(inputs["W2"], np.float32)
    b2 = np.asarray(inputs["b2"], np.float32)
    W_ih = np.asarray(inputs["W_ih"], np.float32)
    W_hh = np.asarray(inputs["W_hh"], np.float32)
    b_ih = np.asarray(inputs["b_ih"], np.float32)
    b_hh = np.asarray(inputs["b_hh"], np.float32)
    fc_w = np.asarray(inputs["fc_w"], np.float32)
    fc_b = np.asarray(inputs["fc_b"], np.float32)

    N = NUM_NODES
    src, dst = ei[0].astype(np.int64), ei[1].astype(np.int64)
    deg = np.bincount(dst, minlength=N).astype(np.float64) + 1.0
    dinv = 1.0 / np.sqrt(deg)
    # A_T[s, d] = normalization weight of edge s->d (plus self loops)
    at = np.zeros((N, N), np.float32)
    np.add.at(at, (src, dst), (dinv[src] * dinv[dst]).astype(np.float32))
    idx = np.arange(N)
    at[idx, idx] += (dinv * dinv).astype(np.float32)
    at16 = at.astype(np.float16)
    del at

    # x natural layout [128, KT*64]
    xpad = np.zeros((NKPAD * KP, IN_FEAT), np.float16)
    xpad[:N] = x.astype(np.float16)
    xn = np.ascontiguousarray(
        xpad[:KT * KP].reshape(KT, KP, IN_FEAT).transpose(1, 0, 2)
    ).reshape(KP, KT * IN_FEAT)

    common = {
        "xn": xn,
        "w1": W1.astype(np.float16),
        "w2": W2.astype(np.float16),
        "wiht": W_ih.T.astype(np.float16),
        "whht": W_hh.T.astype(np.float16),
        "fcwt": fc_w.T.astype(np.float16),
        "ident": np.eye(KP, dtype=np.float16),
        "b1c": b1.reshape(2, KP).T.astype(np.float32).copy(),
        "b2c": b2.reshape(2, KP).T.astype(np.float32).copy(),
        "gib": (b_ih + np.concatenate([b_hh[:2 * HID],
                                       np.zeros(HID, np.float32)])
                ).reshape(6, KP).T.astype(np.float32).copy(),
        "bhn": b_hh[2 * HID:].reshape(2, KP).T.astype(np.float32).copy(),
        "fcb": np.concatenate([fc_b, np.zeros(KP - OUT, np.float32)]
                              ).reshape(KP, 1),
    }

    in_maps = []
    for c in range(CORES):
        r0, r1 = c * ROWS, (c + 1) * ROWS
        strip = np.zeros((NKPAD * KP, L), np.float16)
        if c == 0:
            strip[:N, HALO:] = at16[:, r0:r1]
        else:
            strip[:N, :] = at16[:, r0 - HALO:r1]
        # chunk-major interleave: [(c*NGRP+g)*128+p, j*CW+cc]
        a2t = np.ascontiguousarray(
            strip.reshape(NGRP, GSZ, KP, NCH, CW).transpose(3, 0, 2, 1, 4)
        ).reshape(NCH * NGRP * KP, GSZ * CW)
        patch = np.zeros((KP, 12), np.float32)
        if c == 0:
            # mul=0; add=-60 for r,z gate tiles, 0 for n tiles -> pad cols
            # produce exactly h=0 so row 0 starts from the true h0=0.
            patch[:, 6:10] = -60.0
        else:
            patch[:, 0:6] = 1.0
        in_maps.append({**common, "a2t": a2t, "patch": patch})
    return in_maps


def assemble_output(results):
    outs = [r["out_t"].T for r in results]          # each [ROWS, OUT]
    full = np.concatenate(outs, axis=0).astype(np.float32)
    return full[None]                               # [1, N, OUT]


def kernel(**inputs) -> np.ndarray:
    from concourse import bass_utils

    if "nc" not in _CACHE:
        _CACHE["nc"] = build_program()
    nc = _CACHE["nc"]
    in_maps = host_prepare(inputs)
    res = bass_utils.run_bass_kernel_spmd(
        nc, in_maps, core_ids=list(range(CORES)))
    return assemble_output(res.results)


if __name__ == "__main__":
    import reference

    inputs = {k: np.asarray(v) for k, v in reference.setup_inputs().items()}
    out = kernel(**inputs)
    print("kernel out", out.shape, out.dtype)
    np.save("/root/problem/kernel_out.npy", out)
